# revision 41
# baseline (speedup 1.0000x reference)
"""Trainium2 Bass kernel for nn_MeshUpConv (MeshCNN up-conv block).

Strategy: data-parallel over batch B=8 (one mesh per NeuronCore).

v3 changes vs v2:
  - from_down neighbors host-gathered (pure input rearrangement, same as
    nbup) and shipped chunk-major with centers: conv1 only SWDGE-gathers
    the up-conv half (256ch rows, 3 descs/row instead of 5) and the fdrm
    DRAM bounce is gone; rm1 shrinks to [E, 256].
  - gather protocol leaned out: Tile-managed triggers (count=None) with a
    strict fire-serialization chain (cross-queue concurrent transfers
    corrupt - measured), queue pairs alternating per chunk so desc-gen for
    chunk c+1 lands in drained rings. Measured 13.6us/chunk vs 18.7.
  - feature build: fresh-tile adds (s=a+b) instead of in-place 2a-d stt
    (tensor_tensor ADD is ~2x faster than scalar_tensor_tensor on DVE).
  - all input tensors chunk-major so every per-chunk load is one 2-dim
    contiguous DMA.
"""

import sys

for _p in ("/opt/trn_rl_repo",):
    if _p not in sys.path:
        sys.path.append(_p)

import numpy as np
import ml_dtypes

BF16 = ml_dtypes.bfloat16

B = 8
E_FULL = 16384
CIN = 128
CO = 256
OB = 2          # output channel blocks of 128
EC = 512        # edges per chunk
EPS = 1e-5
GNI = 512       # idxs per dma_gather (hard HW limit; >512 wedges the device)
NQ = 4          # SWDGE queues; chunks alternate pairs (0,1)/(2,3)


def _pack_idx(ei: np.ndarray, E: int) -> np.ndarray:
    """ei [E,4] int32 -> [128, NCH*128] int16 wrapped gather-index layout.

    Per chunk c the 2048 indices are ordered j = s*EC + i (slot-major), and
    index j lives at [16*g + j%16, c*128 + j//16] for every g in 0..7.
    """
    nch = E // EC
    arr = ei.reshape(nch, EC, 4).transpose(0, 2, 1).reshape(nch, 4 * EC)
    w = arr.reshape(nch, (4 * EC) // 16, 16).transpose(2, 0, 1).reshape(16, -1)
    return np.tile(w, (8, 1)).astype(np.int16)


def _pack_w(W: np.ndarray) -> np.ndarray:
    """W [256, C, 5] f32 -> [128, NBLK*128] bf16 lhsT blocks ordered (ob,k,cb)."""
    O, C, K = W.shape
    cb_n = C // 128
    out = np.empty((128, OB * K * cb_n * 128), np.float32)
    n = 0
    for ob in range(OB):
        for k in range(K):
            for cb in range(cb_n):
                blk = W[ob * 128:(ob + 1) * 128, cb * 128:(cb + 1) * 128, k].T
                out[:, n * 128:(n + 1) * 128] = blk
                n += 1
    return out.astype(BF16)


def _pack_b(b: np.ndarray) -> np.ndarray:
    return np.asarray(b).reshape(OB, 128).T.astype(np.float32).copy()


def build_nc(E: int = E_FULL):
    import concourse.bacc as bacc
    import concourse.mybir as mybir
    from concourse.tile import TileContext
    from concourse.tile_rust import add_dep_helper

    dt = mybir.dt
    Alu = mybir.AluOpType
    Act = mybir.ActivationFunctionType
    NCH = E // EC

    nc = bacc.Bacc("TRN2", num_swdge_queues=NQ)

    nbup = nc.dram_tensor("nbup", [NCH, 128, 5 * EC], dt.bfloat16,
                          kind="ExternalInput")  # [c][p][slot(ctr,n1..n4)*EC]
    nfd = nc.dram_tensor("nfd", [NCH, 128, 10 * EC], dt.bfloat16,
                         kind="ExternalInput")  # [c][p][(cb,slot)*EC]
    idx = nc.dram_tensor("idx", [128, NCH * 128], dt.int16, kind="ExternalInput")
    wall = nc.dram_tensor("wall", [128, 90 * 128], dt.bfloat16,
                          kind="ExternalInput")  # wup(10) w1(40) w2a(20) w2b(20)
    bia = nc.dram_tensor("bia", [128, 4 * OB], dt.float32, kind="ExternalInput")
    ident = nc.dram_tensor("ident", [128, 128], dt.bfloat16, kind="ExternalInput")
    out = nc.dram_tensor("out", [CO, E], dt.float32, kind="ExternalOutput")

    rm1 = nc.dram_tensor("rm1", [E, CO], dt.bfloat16, kind="Internal")
    rm2 = nc.dram_tensor("rm2", [E, CO], dt.bfloat16, kind="Internal")
    rm3 = nc.dram_tensor("rm3", [E, CO], dt.bfloat16, kind="Internal")

    # xbar hazard discipline (measured, not folklore):
    #  - concurrent transpose-gather transfers on different queues corrupt
    #    -> serialize fires: before each trigger, Pool waits for the
    #       previously fired queue's transfers (sem chain).
    #  - plain DMA concurrent with gather transfers deadlocks the SDMA
    #    engines -> every plain DMA waits for the last chunk's gather sems
    #    (_wtail) and every trigger waits for plains issued since the last
    #    trigger (_pending).
    _pending = []
    _wtail = []

    def _dma(inst):
        for w in _wtail:
            add_dep_helper(inst.ins, w.ins, reason="dma-after-gather-fence")
        _pending.append(inst)
        return inst

    with TileContext(nc) as tc:
        with (
            tc.tile_pool(name="persist", bufs=1) as persist,
            tc.tile_pool(name="wp", bufs=1) as wpool,
            tc.tile_pool(name="gp", bufs=3) as gpool,
            tc.tile_pool(name="sp", bufs=2) as spool,
            tc.tile_pool(name="cp", bufs=2) as cpool,
            tc.tile_pool(name="bp", bufs=2) as bpool,
            tc.tile_pool(name="jkp", bufs=1) as jkpool,
            tc.tile_pool(name="mmps", bufs=4, space="PSUM") as mmps,
            tc.tile_pool(name="tpps", bufs=2, space="PSUM") as tpps,
        ):
            bufA = persist.tile([128, OB * E], dt.bfloat16, tag="bufA")
            bufB = persist.tile([128, OB * E], dt.bfloat16, tag="bufB")
            id_t = persist.tile([128, 128], dt.bfloat16, tag="ident")
            bias_t = persist.tile([128, 4 * OB], dt.float32, tag="bias")
            ssum = persist.tile([128, OB * NCH], dt.float32, tag="ssum")
            ssq = persist.tile([128, OB * NCH], dt.float32, tag="ssq")
            nrm = persist.tile([128, 8 * OB], dt.float32, tag="nrm")
            ix_all = persist.tile([128, NCH * 128], dt.int16, tag="ix")

            _dma(nc.sync.dma_start(id_t[:], ident[:]))
            _dma(nc.sync.dma_start(bias_t[:], bia[:]))
            _dma(nc.sync.dma_start(ix_all[:], idx[:]))

            gsems = [nc.alloc_semaphore(f"gdma{q}") for q in range(NQ)]
            gstate = {"cnt": [0] * NQ, "lastq": None, "lasttrig": None,
                      "mm_hist": []}

            # ---------------- shared epilogue: transposes -> rm -------------
            def transpose_rows(src_buf, e0, rm_dst):
                """PE-transpose src_buf chunk (both ob) -> rm rows e0..e0+EC.
                The PSUM->SBUF bounce tiles borrow the (idle) spool slots so
                consecutive chunks pipeline without extra SBUF."""
                for ob in range(OB):
                    tp = tpps.tile([128, EC], dt.bfloat16, tag=f"tp{ob}")
                    for g in range(EC // 128):
                        nc.tensor.transpose(
                            tp[:, g * 128:(g + 1) * 128],
                            src_buf[:, ob * E + e0 + g * 128:
                                    ob * E + e0 + g * 128 + 128],
                            id_t[:])
                    rtb = spool.tile([128, EC // 128, 128], dt.bfloat16,
                                     tag=["s13", "s24"][ob])
                    nc.vector.tensor_copy(
                        rtb[:], tp[:].rearrange("p (g n) -> p g n",
                                                g=EC // 128))
                    _dma(nc.sync.dma_start(
                        rm_dst[e0:e0 + EC, ob * 128:(ob + 1) * 128]
                        .rearrange("(g p) c -> p g c", p=128),
                        rtb[:]))

            # ------------------------- up conv ------------------------------
            wu_t = wpool.tile([128, 40 * 128], dt.bfloat16, tag="w")
            _dma(nc.sync.dma_start(wu_t[:, 0:10 * 128], wall[:, 0:10 * 128]))
            for c in range(NCH):
                e0 = c * EC
                nbt = cpool.tile([128, 2, 5, EC], dt.bfloat16, tag="nfd")
                _dma(nc.sync.dma_start(
                    nbt[:, 0, :, :], nbup[c].rearrange("p (s e) -> p s e", s=5)))
                # features: s13=n1+n3, s24=n2+n4 (fresh), d=a-b in place, |d|
                st = spool.tile([128, 2, EC], dt.bfloat16, tag="s13")
                nc.vector.tensor_tensor(st[:, 0, :], nbt[:, 0, 1, :],
                                        nbt[:, 0, 3, :], op=Alu.add)
                nc.vector.tensor_tensor(st[:, 1, :], nbt[:, 0, 2, :],
                                        nbt[:, 0, 4, :], op=Alu.add)
                nc.vector.tensor_tensor(nbt[:, 0, 3, :], nbt[:, 0, 1, :],
                                        nbt[:, 0, 3, :], op=Alu.subtract)
                nc.vector.tensor_tensor(nbt[:, 0, 4, :], nbt[:, 0, 2, :],
                                        nbt[:, 0, 4, :], op=Alu.subtract)
                for s in (3, 4):
                    di = nbt[:, 0, s, :].bitcast(dt.int16)
                    nc.vector.tensor_scalar(di, di, 0x7FFF, None,
                                            op0=Alu.bitwise_and)
                rhs_by_k = [nbt[:, 0, 0, :], st[:, 0, :], st[:, 1, :],
                            nbt[:, 0, 3, :], nbt[:, 0, 4, :]]
                for ob in range(OB):
                    ps = mmps.tile([128, EC], dt.float32, tag="ps")
                    for k in range(5):
                        n = ob * 5 + k
                        nc.tensor.matmul(
                            ps[:], wu_t[:, n * 128:(n + 1) * 128], rhs_by_k[k],
                            start=(k == 0), stop=(k == 4))
                    nc.scalar.activation(
                        bufB[:, ob * E + e0:ob * E + e0 + EC], ps[:],
                        Act.Identity, bias=bias_t[:, ob:ob + 1])
                transpose_rows(bufB, e0, rm1)

            # ------------------------ conv pass -----------------------------
            def conv_pass(table, w_off, nblk, bias_col, center_fn, fdf_fn,
                          pre_chunk=None):
                """One mesh-conv pass: SWDGE-gather 4 slots x 256ch from
                `table`, build features, matmul into bufA + stats.

                Queue assignment by slot-pair: (n1,n3) on one queue, (n2,n4)
                on the other, so the s13/a13 features only wait for the
                first queue's transfers.
                Matmul k-order [3,4,1,2,0]: gather-tile readers first (frees
                the WAR on the next chunk's preps early), centers last.
                """
                w_t = wpool.tile([128, 40 * 128], dt.bfloat16, tag="w")
                _dma(nc.sync.dma_start(
                    w_t[:, 0:nblk * 128],
                    wall[:, w_off * 128:(w_off + nblk) * 128]))
                for c in range(NCH):
                    e0 = c * EC
                    qs = [(2 * c) % NQ, (2 * c + 1) % NQ]
                    gts = [None] * 4
                    wvs = []
                    # slot-pair queues: (n1,n3) on qs[0], (n2,n4) on qs[1];
                    # max 2 untriggered preps per ring (>2 corrupts - the
                    # carveout reclaim trusts sem bookkeeping it can't have).
                    for j, pair in enumerate(((0, 2), (1, 3))):
                        q = qs[j]
                        for g in pair:
                            gt = gpool.tile([128, 2, EC], dt.bfloat16,
                                            tag=f"gd{g}")
                            nc.gpsimd.dma_gather(
                                gt[:], table[:],
                                ix_all[:, c * 128 + g * 32:
                                       c * 128 + (g + 1) * 32],
                                num_idxs=GNI, num_idxs_reg=GNI,
                                elem_size=CO, transpose=True, queue_num=q,
                                prepare_only=True, sem=gsems[q],
                            )
                            gts[g] = gt
                        # serialize fires: wait for the previously fired
                        # queue's transfers before ringing this doorbell
                        prev = gstate["lasttrig"]
                        if gstate["lastq"] is not None:
                            qp = gstate["lastq"]
                            w = nc.gpsimd.wait_ge(gsems[qp],
                                                  16 * gstate["cnt"][qp])
                            if prev is not None:
                                add_dep_helper(w.ins, prev.ins,
                                               reason="trig-chain")
                            prev = w
                        t = nc.gpsimd.trigger_dma(count=None, queue_num=q)
                        if prev is not None:
                            add_dep_helper(t.ins, prev.ins, reason="trig-chain")
                        if j == 0:
                            # plain-DMA exclusion + table RAW: first fire of
                            # the chunk waits for all plains since last fire.
                            for d in _pending:
                                add_dep_helper(t.ins, d.ins,
                                               reason="gather-fence")
                            _pending.clear()
                        # WAR: transfers overwrite tiles the
                        # chunk-before-last's matmuls read.
                        if len(gstate["mm_hist"]) >= 2:
                            add_dep_helper(t.ins, gstate["mm_hist"][-2].ins,
                                           reason="gather-war")
                        gstate["cnt"][q] += 2
                        gstate["lastq"] = q
                        gstate["lasttrig"] = t
                        # RAW: consumers wait for this half's transfers
                        wv = nc.vector.wait_ge(gsems[q], 16 * gstate["cnt"][q])
                        add_dep_helper(wv.ins, t.ins, reason="wait-after-trig")
                        wvs.append(wv)
                    _wtail.clear()
                    _wtail.extend(wvs)
                    if pre_chunk:
                        pre_chunk(c)
                    # features; pair (n1,n3)=gts[0],gts[2] needs wvs[0] only
                    st = spool.tile([128, 2, EC], dt.bfloat16, tag="s13")
                    su = spool.tile([128, 2, EC], dt.bfloat16, tag="s24")
                    f1 = nc.vector.tensor_tensor(st[:], gts[0][:], gts[2][:],
                                                 op=Alu.add)
                    f3 = nc.vector.tensor_tensor(gts[2][:], gts[0][:],
                                                 gts[2][:], op=Alu.subtract)
                    d2 = gts[2][:].bitcast(dt.int16)
                    nc.vector.tensor_scalar(d2, d2, 0x7FFF, None,
                                            op0=Alu.bitwise_and)
                    f2 = nc.vector.tensor_tensor(su[:], gts[1][:], gts[3][:],
                                                 op=Alu.add)
                    f4 = nc.vector.tensor_tensor(gts[3][:], gts[1][:],
                                                 gts[3][:], op=Alu.subtract)
                    d3 = gts[3][:].bitcast(dt.int16)
                    nc.vector.tensor_scalar(d3, d3, 0x7FFF, None,
                                            op0=Alu.bitwise_and)
                    for f in (f1, f3):
                        add_dep_helper(f.ins, wvs[0].ins, reason="gather-raw")
                    for f in (f2, f4):
                        add_dep_helper(f.ins, wvs[1].ins, reason="gather-raw")
                    ctrs = center_fn(c)
                    fdf = fdf_fn(c) if fdf_fn else None
                    CB = len(ctrs)
                    last_mm = None
                    for ob in range(OB):
                        ps = mmps.tile([128, EC], dt.float32, tag="ps")
                        nmm = 5 * CB
                        i_mm = 0
                        for k in (3, 4, 1, 2, 0):
                            for cb in range(CB):
                                if k == 0:
                                    rhs = ctrs[cb]
                                elif cb < 2:
                                    rhs = [st, su, gts[2], gts[3]][k - 1][
                                        :, cb, :]
                                else:
                                    rhs = fdf[k - 1][:, cb - 2, :]
                                n = (ob * 5 + k) * CB + cb
                                mm = nc.tensor.matmul(
                                    ps[:], w_t[:, n * 128:(n + 1) * 128], rhs,
                                    start=(i_mm == 0), stop=(i_mm == nmm - 1),
                                )
                                if k in (3, 4):
                                    last_mm = mm
                                i_mm += 1
                        raw_ap = bufA[:, ob * E + e0:ob * E + e0 + EC]
                        nc.scalar.activation(
                            raw_ap, ps[:], Act.Identity,
                            bias=bias_t[:, bias_col * OB + ob:
                                        bias_col * OB + ob + 1],
                            accum_out=ssum[:, ob * NCH + c:ob * NCH + c + 1],
                        )
                        jk = jkpool.tile([128, EC], dt.bfloat16, tag="jk")
                        nc.scalar.activation(
                            jk[:], raw_ap, Act.Square,
                            accum_out=ssq[:, ob * NCH + c:ob * NCH + c + 1],
                        )
                    gstate["mm_hist"].append(last_mm)

            # -------------------- stats finalize ---------------------------
            def conv_finalize(slot):
                mean = nrm[:, 0:OB]
                var = nrm[:, OB:2 * OB]
                scal = nrm[:, (2 + 2 * slot) * OB:(3 + 2 * slot) * OB]
                shift = nrm[:, (3 + 2 * slot) * OB:(4 + 2 * slot) * OB]
                for ob in range(OB):
                    nc.vector.reduce_sum(
                        mean[:, ob:ob + 1], ssum[:, ob * NCH:(ob + 1) * NCH],
                        axis=mybir.AxisListType.X)
                    nc.vector.reduce_sum(
                        var[:, ob:ob + 1], ssq[:, ob * NCH:(ob + 1) * NCH],
                        axis=mybir.AxisListType.X)
                nc.vector.tensor_scalar(mean, mean, 1.0 / E, None, op0=Alu.mult)
                nc.vector.tensor_scalar(var, var, 1.0 / E, None, op0=Alu.mult)
                nc.vector.scalar_tensor_tensor(
                    shift, mean, -1.0, mean, op0=Alu.mult, op1=Alu.mult)
                nc.vector.tensor_tensor(var, var, shift, op=Alu.add)
                nc.vector.tensor_scalar(var, var, EPS, None, op0=Alu.add)
                nc.scalar.activation(var, var, Act.Sqrt)
                nc.vector.reciprocal(scal, var)
                nc.vector.scalar_tensor_tensor(
                    shift, mean, -1.0, scal, op0=Alu.mult, op1=Alu.mult)
                return scal, shift

            # ------------------------- conv1 --------------------------------
            c1_tiles = {}

            def c1_load(c):
                nf = cpool.tile([128, 2, 5, EC], dt.bfloat16, tag="nfd")
                _dma(nc.sync.dma_start(
                    nf[:], nfd[c].rearrange("p (b s e) -> p b s e", b=2, s=5)))
                c1_tiles[c] = nf

            def c1_pre(c):
                if c + 1 < NCH:
                    c1_load(c + 1)

            def c1_center(c):
                e0 = c * EC
                nf = c1_tiles.pop(c)
                # fd features fully in place: d=a-b into b, s=2a-d into a, |d|
                nc.vector.tensor_tensor(nf[:, :, 3, :], nf[:, :, 1, :],
                                        nf[:, :, 3, :], op=Alu.subtract)
                nc.vector.tensor_tensor(nf[:, :, 4, :], nf[:, :, 2, :],
                                        nf[:, :, 4, :], op=Alu.subtract)
                nc.vector.scalar_tensor_tensor(
                    nf[:, :, 1, :], nf[:, :, 1, :], 2.0, nf[:, :, 3, :],
                    op0=Alu.mult, op1=Alu.subtract)
                nc.vector.scalar_tensor_tensor(
                    nf[:, :, 2, :], nf[:, :, 2, :], 2.0, nf[:, :, 4, :],
                    op0=Alu.mult, op1=Alu.subtract)
                for s in (3, 4):
                    di = nf[:, :, s, :].bitcast(dt.int16)
                    nc.vector.tensor_scalar(di, di, 0x7FFF, None,
                                            op0=Alu.bitwise_and)
                c1_state["fdf"] = (nf[:, :, 1, :], nf[:, :, 2, :],
                                   nf[:, :, 3, :], nf[:, :, 4, :])
                return [bufB[:, 0 * E + e0:0 * E + e0 + EC],
                        bufB[:, 1 * E + e0:1 * E + e0 + EC],
                        nf[:, 0, 0, :], nf[:, 1, 0, :]]

            c1_state = {}
            c1_load(0)
            conv_pass(rm1, 10, 40, 1, c1_center, lambda c: c1_state["fdf"],
                      pre_chunk=c1_pre)

            # epi1: x1n = relu(norm(raw1)) -> bufB ; transposes -> rm2
            # (ob0 on Act, ob1 on DVE - halves the serial Act stream)
            scal, shift = conv_finalize(0)
            for c in range(NCH):
                e0 = c * EC
                nc.scalar.activation(
                    bufB[:, e0:e0 + EC], bufA[:, e0:e0 + EC],
                    Act.Relu, bias=shift[:, 0:1], scale=scal[:, 0:1])
                b1 = bufB[:, E + e0:E + e0 + EC]
                nc.vector.tensor_scalar(
                    b1, bufA[:, E + e0:E + e0 + EC],
                    scal[:, 1:2], shift[:, 1:2], op0=Alu.mult, op1=Alu.add)
                nc.vector.tensor_scalar(b1, b1, 0.0, None, op0=Alu.max)
                transpose_rows(bufB, e0, rm2)

            # ------------------------- conv2a -------------------------------
            def c2_center(c):
                e0 = c * EC
                return [bufB[:, 0 * E + e0:0 * E + e0 + EC],
                        bufB[:, 1 * E + e0:1 * E + e0 + EC]]

            conv_pass(rm2, 50, 20, 2, c2_center, None)

            # epi2: x2 = relu(norm(raw2a) + x1n) -> bufB ; transposes -> rm3
            scal, shift = conv_finalize(1)
            for c in range(NCH):
                e0 = c * EC
                for ob in range(OB):
                    t = bpool.tile([128, EC], dt.bfloat16, tag="bt")
                    nc.scalar.activation(
                        t[:], bufA[:, ob * E + e0:ob * E + e0 + EC],
                        Act.Identity, bias=shift[:, ob:ob + 1],
                        scale=scal[:, ob:ob + 1])
                    nc.vector.tensor_tensor(
                        t[:], t[:], bufB[:, ob * E + e0:ob * E + e0 + EC],
                        op=Alu.add)
                    nc.vector.tensor_scalar(
                        bufB[:, ob * E + e0:ob * E + e0 + EC], t[:], 0.0, None,
                        op0=Alu.max)
                transpose_rows(bufB, e0, rm3)

            # ------------------------- conv2b -------------------------------
            conv_pass(rm3, 70, 20, 3, c2_center, None)

            # epi3: out = relu(norm(raw2b) + x2) -> DRAM f32
            scal, shift = conv_finalize(2)
            for c in range(NCH):
                e0 = c * EC
                for ob in range(OB):
                    t = bpool.tile([128, EC], dt.bfloat16, tag="bt")
                    nc.scalar.activation(
                        t[:], bufA[:, ob * E + e0:ob * E + e0 + EC],
                        Act.Identity, bias=shift[:, ob:ob + 1],
                        scale=scal[:, ob:ob + 1])
                    u = bpool.tile([128, EC], dt.float32, tag="ut")
                    nc.vector.tensor_tensor(
                        u[:], t[:], bufB[:, ob * E + e0:ob * E + e0 + EC],
                        op=Alu.add)
                    nc.vector.tensor_scalar(u[:], u[:], 0.0, None, op0=Alu.max)
                    _dma(nc.sync.dma_start(
                        out[ob * 128:(ob + 1) * 128, e0:e0 + EC], u[:]))

    nc.finalize()
    return nc


_NC_CACHE = {}


def _get_nc(E):
    if E not in _NC_CACHE:
        _NC_CACHE[E] = build_nc(E)
    return _NC_CACHE[E]


def make_in_maps(from_up, from_down, edge_index, W_up, b_up, W1, b1, W2a, b2a,
                 W2b, b2b, E=E_FULL):
    """Build the per-core input maps (host-side sharding + layout packing)."""
    NCH = E // EC
    w_all = np.concatenate(
        [_pack_w(np.asarray(W_up)), _pack_w(np.asarray(W1)),
         _pack_w(np.asarray(W2a)), _pack_w(np.asarray(W2b))], axis=1)
    bia_p = np.concatenate(
        [_pack_b(b_up), _pack_b(b1), _pack_b(b2a), _pack_b(b2b)], axis=1)
    ident = np.eye(128, dtype=BF16)
    in_maps = []
    for i in range(B):
        fu_b = np.asarray(from_up[i], np.float32).astype(BF16)       # [128,E]
        fd_b = np.asarray(from_down[i], np.float32).astype(BF16)     # [256,E]
        ei = np.asarray(edge_index[i])                               # [E,4]
        # nbup [c][p][slot*EC]: slot 0 centers, 1..4 neighbors
        nb = np.empty((128, 5, E), BF16)
        nb[:, 0] = fu_b
        for s in range(4):
            nb[:, s + 1] = fu_b[:, ei[:, s]]
        nbup = np.ascontiguousarray(
            nb.reshape(128, 5, NCH, EC).transpose(2, 0, 1, 3)
        ).reshape(NCH, 128, 5 * EC)
        # nfd [c][p][(cb,slot)*EC]
        nf = np.empty((128, 2, 5, E), BF16)
        fd3 = fd_b.reshape(2, 128, E).transpose(1, 0, 2)             # [128,2,E]
        nf[:, :, 0] = fd3
        for s in range(4):
            nf[:, :, s + 1] = fd3[:, :, ei[:, s]]
        nfd = np.ascontiguousarray(
            nf.reshape(128, 2, 5, NCH, EC).transpose(3, 0, 1, 2, 4)
        ).reshape(NCH, 128, 10 * EC)
        in_maps.append({
            "nbup": nbup,
            "nfd": nfd,
            "idx": _pack_idx(ei, E),
            "wall": w_all,
            "bia": bia_p, "ident": ident,
        })
    return in_maps


def kernel(from_up, from_down, edge_index, W_up, b_up, W1, b1, W2a, b2a,
           W2b, b2b) -> np.ndarray:
    from concourse import bass_utils

    nc = _get_nc(E_FULL)
    in_maps = make_in_maps(from_up, from_down, edge_index, W_up, b_up,
                           W1, b1, W2a, b2a, W2b, b2b)
    res = bass_utils.run_bass_kernel_spmd(nc, in_maps, core_ids=list(range(B)))
    return np.stack([r["out"] for r in res.results]).astype(np.float32)
